# revision 54
# baseline (speedup 1.0000x reference)
"""Trainium2 Bass kernel for an AttentionBlock (1x1-conv QKV + softmax attention + residual).

Reference computation (per batch b):
    q = Wq@x + bq  [32, N];  k = Wk@x + bk  [32, N];  v = Wv@x + bv  [256, N]
    attn = softmax_j(q_i . k_j);  out[c, i] = sum_j v[c, j] attn[i, j]
    final = gamma * out + x            (N = 64*64 = 4096)

Sharding: 8 cores = 4 batches x 2 query-halves (2048 queries per core).
Each core receives x[b] with its columns rolled so its own query half sits at
columns 0:2048 (softmax is invariant to a permutation of the key/value axis).

Per-core device program (bf16 datapath; fp32 only for PSUM accum, the exp
input, the softmax reciprocal and the residual add, so the gamma=0 path
returns x exactly):
    x arrives bf16 in 8 column chunks on one DMA queue (in-order arrival;
    separate tiles, so projections start as soon as chunk 0 lands). Dummy
    warm-up matmuls keep the PE busy meanwhile so it reaches 2.4 GHz.
    k[128, 4096], q[128, 2048] = W4 @ x + bias: weights are band-replicated
    4x on the host, so the projection itself fills all four 32-row bands
    (no replication DMAs); bias-adds alternate between DVE and ACT.
    vT[j][128, 257] per 128-key tile = (gamma*Wv) @ x; col 256 = 1.0 (the
    softmax denominator rides the attention matmul as an extra channel;
    gamma*bv collapses into the residual since softmax rows sum to 1).
    scores quad [128k, 4, 512q] = 4 concurrent 32-row tile_position MMs
    e = exp(scores - 40) -> bf16, one ACTIVATE per quad (N=2048, the exp
    chain is the pipeline backbone at ~2.66us/step)
    out[q, 0:257] += e-tile.T @ vT[j]      (PE, bf16, ~110ns/MM roofline)
    final[i, c] = out[i, c]/denom_i + (xT[i, c] + gamma*bv_c)
Output is stored [n, c]; the host transposes back to [c, n].
"""

import sys

if "/opt/trn_rl_repo" not in sys.path:
    sys.path.insert(0, "/opt/trn_rl_repo")

import numpy as np

import concourse.bass as bass
import concourse.tile as tile
from concourse import bacc
from concourse import mybir

F32 = mybir.dt.float32
BF16 = mybir.dt.bfloat16

C = 256          # channels
D = 32           # q/k channels
NK = 4096        # keys per core (full sequence)
NQ = 2048        # queries per core (half sequence)
NJ = NK // 128   # 32 key tiles
NG = 4           # query groups
GI = 4           # i-tiles (128 queries) per group
ISPAN = NQ // NG  # 512 query columns per group
NCH = 8          # x column chunks of 512
EXP_SHIFT = -40.0

Exp = mybir.ActivationFunctionType.Exp
Ident = mybir.ActivationFunctionType.Identity


# params_bf column layout (per partition p = one of 128 input-channel rows):
#   0:256    W4k  (h*128 + 32r + d)  -- Wk.T band-replicated 4x along M
#   256:512  W4q
#   512:1024 wv   (h*256 + c)
PW_K, PW_Q, PW_V = 0, 256, 512
PBF_COLS = 1024


def build(nc):
    x_bf = nc.declare_dram_parameter("x_bf", [C, NK], BF16, isOutput=False)
    xqT = nc.declare_dram_parameter("xqT", [NQ, C], F32, isOutput=False)
    params_bf = nc.declare_dram_parameter("params_bf", [128, PBF_COLS], BF16, isOutput=False)
    params_f32 = nc.declare_dram_parameter("params_f32", [128, 3], F32, isOutput=False)
    out_nc = nc.declare_dram_parameter("out_nc", [NQ, C], F32, isOutput=True)

    with tile.TileContext(nc) as tc:
        with (
            tc.tile_pool(name="singles", bufs=1) as singles,
            tc.tile_pool(name="epool", bufs=9) as e_pool,
            tc.tile_pool(name="osb", bufs=3) as osb_pool,
            tc.tile_pool(name="small", bufs=8) as small_pool,
            tc.tile_pool(name="s_ps", bufs=1, space="PSUM") as s_pool,
            tc.tile_pool(name="o_ps", bufs=4, space="PSUM") as o_pool,
        ):
            # ---------------- persistent SBUF inputs ----------------
            # All small weights ride ONE fast HWDGE transfer each; only the
            # late-needed xqT rides the slow gpsimd SWDGE path.
            pbf = singles.tile([128, PBF_COLS], BF16, name="params_bf")
            nc.scalar.dma_start(out=pbf, in_=params_bf[:, :])
            pf32 = singles.tile([128, 3], F32, name="params_f32")
            nc.scalar.dma_start(out=pf32, in_=params_f32[:, :])
            bk4_sb = pf32[:, 0:1]
            bq4_sb = pf32[:, 1:2]
            gamma_sb = pf32[:, 2:3]

            shift_sb = singles.tile([128, 1], F32)
            nc.vector.memset(shift_sb, EXP_SHIFT)

            # PE warm-up: dummy matmuls on memset data keep the PE busy from
            # program start until x chunk 0 lands (~4.5us), which both trips
            # the HAM un-throttle (needs ~3.4us of sustained busy) and avoids
            # the idle window that would re-throttle it -- so the projection
            # matmuls run at 2.4 GHz instead of 1.2.
            wu_src = singles.tile([128, 2, 512], BF16, name="wu")
            with tc.high_priority():
                nc.vector.memset(wu_src, 0.0)
                wu_ps = s_pool.tile([128, 4, ISPAN], F32, tag="ps_s", name="wu_ps")
                for i in range(6):
                    nc.tensor.matmul(
                        wu_ps[:, i % 4, :], wu_src[:, 0, 0:128], wu_src[:, 1, :],
                        start=True, stop=True,
                    )

            # x in 8 column chunks, ALL on the sync queue: within one queue
            # the descriptors drain in order, so chunk 0 completes ~0.7us
            # after issue instead of finishing together with all 2MB (the
            # SDMA engines round-robin across queues at packet granularity,
            # so splitting across queues destroys arrival ordering).
            x_r = x_bf.rearrange("(h p) n -> p h n", p=128)
            x_ch = [None] * NCH
            for cch in range(NCH):
                t = singles.tile([128, 2, 512], BF16, name=f"x{cch}")
                nc.sync.dma_start(out=t, in_=x_r[:, :, cch * 512 : (cch + 1) * 512])
                x_ch[cch] = t

            # xqT (residual, needed only at epilogues) queues behind the x
            # chunks so it never competes with them for HBM bandwidth.
            xqT_sb = singles.tile([128, NQ // 128, C], F32)
            nc.sync.dma_start(
                out=xqT_sb, in_=xqT.rearrange("(t p) c -> p t c", p=128)
            )

            # ---------------- k/q projections ----------------
            # Weights are band-replicated 4x on the host (W4), so one matmul
            # fills all four 32-row bands of k/q -- no replication DMAs.
            k_h = [
                singles.tile([128, NK // 2], BF16, name="k_h0"),
                singles.tile([128, NK // 2], BF16, name="k_h1"),
            ]
            q_sb = singles.tile([128, NQ], BF16)

            def kq_proj(w_off, b_sb, dst, dst_off, cch, slot):
                # one x chunk -> two 256-col psum slices -> bf16, all 128 rows.
                # Bias-adds alternate between DVE and the idle ACT engine.
                for s in range(2):
                    ps = o_pool.tile([128, C + 2], F32, tag="ps_o", name="ps_kq")
                    for h in range(2):
                        nc.tensor.matmul(
                            ps[:, 0:256],
                            pbf[:, w_off + h * 128 : w_off + (h + 1) * 128],
                            x_ch[cch][:, h, s * 256 : (s + 1) * 256],
                            start=(h == 0),
                            stop=(h == 1),
                        )
                    dsl = dst[:, dst_off + s * 256 : dst_off + (s + 1) * 256]
                    if (slot + s) % 2 == 0:
                        nc.vector.tensor_scalar_add(dsl, ps[:, 0:256], b_sb)
                    else:
                        nc.scalar.activation(
                            dsl, ps[:, 0:256], Ident, bias=b_sb, scale=1.0
                        )

            def kq_extra(m):
                # late k/q chunks, interleaved into the v-proj loop just
                # ahead of their first consumer
                if m < 4:
                    kq_proj(PW_K, bk4_sb, k_h[1], m * 512, m + 4, 0)
                elif m < 7:
                    kq_proj(PW_Q, bq4_sb, q_sb, (m - 3) * 512, m - 3, 1)

            # ---------------- v projection (per 128-key tile) ----------------
            # No v-bias on device: softmax weights sum to 1, so attn@(v+bv) =
            # attn@v + bv and the host folds gamma*bv into the residual xqT.
            # Column 256 of each vT tile (the softmax-denominator ones column)
            # is pre-memset here while the DVE is otherwise idle.
            vT = []
            for j in range(NJ):
                t = singles.tile([128, C + 1], BF16, name=f"vT{j}")
                nc.vector.memset(t[:, C : C + 1], 1.0)
                vT.append(t)

            def v_proj(j):
                cch, lj = j // 4, j % 4
                psv = o_pool.tile([128, C + 2], F32, tag="ps_o", name="ps_v")
                for h in range(2):
                    nc.tensor.matmul(
                        psv[:, 0:C],
                        x_ch[cch][:, h, lj * 128 : (lj + 1) * 128],
                        pbf[:, PW_V + h * C : PW_V + (h + 1) * C],
                        start=(h == 0),
                        stop=(h == 1),
                    )
                nc.vector.tensor_copy(vT[j][:, 0:C], psv[:, 0:C])

            # ---------------- attention ----------------
            steps = [(g, q4) for g in range(NG) for q4 in range(NJ // 4)]
            score_tiles = {}

            def emit_scores(step):
                g, q4 = step
                kh = k_h[q4 // 4]
                base = (q4 % 4) * 512
                ps_s = s_pool.tile([128, 4, ISPAN], F32, tag="ps_s", name="ps_s")
                for r in range(4):
                    nc.tensor.matmul(
                        ps_s[:, r, :],
                        kh[32 * r : 32 * (r + 1), base + r * 128 : base + (r + 1) * 128],
                        q_sb[32 * r : 32 * (r + 1), g * ISPAN : (g + 1) * ISPAN],
                        start=True,
                        stop=True,
                        tile_position=(32 * r, 0),
                    )
                e_sb = e_pool.tile([128, 4, ISPAN], BF16, tag="e_sb", name="e_sb")
                nc.scalar.activation(e_sb, ps_s, Exp, bias=shift_sb, scale=1.0)
                score_tiles[step] = e_sb

            def emit_tile_epilogue(g, t, ps_o):
                # gamma rides in Wv (host-folded), so f = ps_o/denom + xqT:
                # one reciprocal + one fused multiply-add, and the PSUM
                # accumulator frees as early as possible.
                it = g * GI + t
                r = small_pool.tile([128, 1], F32, tag="r", name="r")
                nc.vector.reciprocal(r, ps_o[t][:, C : C + 1])
                f_sb = osb_pool.tile([128, C], F32, tag="f_sb", name="f_sb")
                nc.vector.scalar_tensor_tensor(
                    f_sb,
                    ps_o[t][:, 0:C],
                    r,
                    xqT_sb[:, it, :],
                    op0=mybir.AluOpType.mult,
                    op1=mybir.AluOpType.add,
                )
                nc.sync.dma_start(out=out_nc[it * 128 : (it + 1) * 128, :], in_=f_sb)

            def emit_attn(step, ps_o):
                g, q4 = step
                e_sb = score_tiles.pop(step)
                if q4 == NJ // 4 - 1:
                    # last step of the group: finish tile-by-tile so each PSUM
                    # accumulator's epilogue starts (and the buffer frees for
                    # the next group) without waiting for the other tiles.
                    for t in range(GI):
                        for r in range(4):
                            j = q4 * 4 + r
                            nc.tensor.matmul(
                                ps_o[t][:, 0 : C + 1],
                                e_sb[:, r, t * 128 : (t + 1) * 128],
                                vT[j],
                                start=False,
                                stop=(j == NJ - 1),
                            )
                        emit_tile_epilogue(g, t, ps_o)
                elif q4 == 0:
                    # first step of a group: tile-major order, so tile t's
                    # first matmul waits only on tile t's buffer -- the
                    # previous group's epilogues free them one by one.
                    for t in range(GI):
                        for r in range(4):
                            nc.tensor.matmul(
                                ps_o[t][:, 0 : C + 1],
                                e_sb[:, r, t * 128 : (t + 1) * 128],
                                vT[r],
                                start=(r == 0),
                                stop=False,
                            )
                else:
                    for r in range(4):
                        j = q4 * 4 + r
                        for t in range(GI):
                            nc.tensor.matmul(
                                ps_o[t][:, 0 : C + 1],
                                e_sb[:, r, t * 128 : (t + 1) * 128],
                                vT[j],
                                start=(j == 0),
                                stop=(j == NJ - 1),
                            )

            # Group 0's scores need only q chunk 0 plus k chunk q4, so each of
            # the first four quads is emitted right behind the k-chunk it
            # consumes -- the exp chain starts as soon as chunk 0 lands.
            # The v-projections and remaining k/q chunks interleave after,
            # giving the ACT engine a LEAD-deep runway of e tiles.
            LEAD = 6
            with tc.high_priority():
                kq_proj(PW_K, bk4_sb, k_h[0], 0, 0, 0)
                kq_proj(PW_Q, bq4_sb, q_sb, 0, 0, 1)
                emit_scores(steps[0])
            for cch in range(1, 4):
                kq_proj(PW_K, bk4_sb, k_h[0], cch * 512, cch, 0)
                with tc.high_priority():
                    emit_scores(steps[cch])
            for m in range(8):
                v_proj(4 * m)
                v_proj(4 * m + 1)
                kq_extra(m)
                if m < 2:
                    emit_scores(steps[m + 4])
                v_proj(4 * m + 2)
                v_proj(4 * m + 3)
            # In-loop score emission runs at lead 2 (the PE parks on the next
            # pair right as its exp dependency clears); the LEAD-8 prefill
            # above is consumed over the first six steps so attention never
            # lags the exp chain by more than ~2 steps.
            ps_o_g = None
            for idx, (g, q4) in enumerate(steps):
                if idx + LEAD < len(steps):
                    emit_scores(steps[idx + LEAD])
                if q4 == 0:
                    ps_o_g = [
                        o_pool.tile([128, C + 2], F32, tag="ps_o", name="ps_o")
                        for _ in range(GI)
                    ]
                emit_attn((g, q4), ps_o_g)
    return nc


def _install_trace_support():
    """Profiling-only plumbing for KERNEL_TRACE=1 runs: register the NTFF
    profile hook (this image's antenv lacks the axon_hooks shim) and keep
    trace artifacts local instead of uploading. Never used in plain runs."""
    import importlib.util
    import types

    import concourse.bass_utils as bu

    bu.upload_artifacts = lambda tmpdir: tmpdir
    if "antenv.axon_hooks" in sys.modules:
        return
    try:
        if importlib.util.find_spec("antenv.axon_hooks") is not None:
            return
    except (ValueError, ModuleNotFoundError):
        return
    import antenv
    from trn_agent_boot.trn_boot import _ntff_profile_via_ctypes

    mod = types.ModuleType("antenv.axon_hooks")
    mod._hook = _ntff_profile_via_ctypes("/opt/axon/libaxon_pjrt.so")
    mod.set_axon_ntff_profile_hook = lambda h: setattr(mod, "_hook", h)
    mod.get_axon_ntff_profile_hook = lambda: mod._hook
    sys.modules["antenv.axon_hooks"] = mod
    antenv.axon_hooks = mod


_cached = None


def _get_module():
    global _cached
    if _cached is None:
        nc = bacc.Bacc()
        build(nc)
        if not nc.is_finalized():
            nc.finalize()
        _cached = nc
    return _cached


def kernel(x, Wq, bq, Wk, bk, Wv, bv, gamma, **_unused):
    from concourse.bass_utils import run_bass_kernel_spmd
    import os

    import ml_dtypes

    B, Cx, H, W = x.shape
    N = H * W
    xf = np.ascontiguousarray(np.asarray(x, dtype=np.float32).reshape(B, Cx, N))
    Wq = np.asarray(Wq, np.float32)
    Wk = np.asarray(Wk, np.float32)
    Wv = np.asarray(Wv, np.float32)
    bq = np.asarray(bq, np.float32)
    bk = np.asarray(bk, np.float32)
    bv = np.asarray(bv, np.float32)
    gamma = np.asarray(gamma, np.float32)

    # params_bf blob: see layout comment above build()
    pblob = np.zeros((128, PBF_COLS), np.float32)
    for h in range(2):
        for r in range(4):
            # W4k[p, h*128 + 32r + d] = Wk[d, h*128 + p]
            pblob[:, PW_K + h * 128 + 32 * r : PW_K + h * 128 + 32 * r + 32] = Wk[
                :, h * 128 : (h + 1) * 128
            ].T
            pblob[:, PW_Q + h * 128 + 32 * r : PW_Q + h * 128 + 32 * r + 32] = Wq[
                :, h * 128 : (h + 1) * 128
            ].T
        # wv[p, h*256 + c] = gamma*Wv[c, h*128 + p]  (gamma folded into Wv;
        # the softmax-denominator ones column stays unscaled)
        pblob[:, PW_V + h * C : PW_V + (h + 1) * C] = (
            gamma[0] * Wv[:, h * 128 : (h + 1) * 128].T
        )
    pblob_bf = np.ascontiguousarray(pblob.astype(ml_dtypes.bfloat16))
    pf32 = np.zeros((128, 3), np.float32)
    pf32[:, 0] = np.tile(bk, 4)
    pf32[:, 1] = np.tile(bq, 4)
    pf32[:, 2] = gamma[0]
    pf32 = np.ascontiguousarray(pf32)

    in_maps = []
    for core in range(8):
        b, half = core // 2, core % 2
        ioff = half * NQ
        xb = xf[b]
        x_roll = np.roll(xb, -ioff, axis=1)
        x_bf = np.ascontiguousarray(x_roll.astype(ml_dtypes.bfloat16))
        # residual + gamma*bv: softmax rows sum to 1, so the v-bias reduces to
        # a constant channel offset folded into the residual tensor.
        xqT_np = np.ascontiguousarray(xb[:, ioff : ioff + NQ].T + gamma[0] * bv[None, :])
        in_maps.append(
            {
                "x_bf": x_bf,
                "xqT": xqT_np,
                "params_bf": pblob_bf,
                "params_f32": pf32,
            }
        )

    nc = _get_module()
    trace = bool(int(os.environ.get("KERNEL_TRACE", "0")))
    if trace:
        _install_trace_support()
        tmpdir = os.environ.get("KERNEL_TRACE_DIR") or None
        res = run_bass_kernel_spmd(
            nc, in_maps, core_ids=list(range(8)), trace=True, tmpdir=tmpdir
        )
    else:
        res = run_bass_kernel_spmd(nc, in_maps, core_ids=list(range(8)))
    if trace and res.exec_time_ns is not None:
        print(f"HW exec time: {res.exec_time_ns} ns")
        print(f"HW exec time mean: {res.mean_exec_time_ns} ns")
        if res.instructions_and_trace is not None:
            print(f"trace: {res.instructions_and_trace[1]}")

    out = np.empty((B, Cx, N), np.float32)
    for core in range(8):
        b, half = core // 2, core % 2
        out[b][:, half * NQ : (half + 1) * NQ] = res.results[core]["out_nc"].T
    return out.reshape(B, Cx, H, W)


# revision 56
# speedup vs baseline: 1.0208x; 1.0208x over previous
"""Trainium2 Bass kernel for an AttentionBlock (1x1-conv QKV + softmax attention + residual).

Reference computation (per batch b):
    q = Wq@x + bq  [32, N];  k = Wk@x + bk  [32, N];  v = Wv@x + bv  [256, N]
    attn = softmax_j(q_i . k_j);  out[c, i] = sum_j v[c, j] attn[i, j]
    final = gamma * out + x            (N = 64*64 = 4096)

Sharding: 8 cores = 4 batches x 2 query-halves (2048 queries per core).
Each core receives x[b] with its columns rolled so its own query half sits at
columns 0:2048 (softmax is invariant to a permutation of the key/value axis).

Per-core device program (bf16 datapath; fp32 only for PSUM accum, the exp
input, the softmax reciprocal and the residual add, so the gamma=0 path
returns x exactly):
    x arrives bf16 in 8 column chunks on one DMA queue (in-order arrival;
    separate tiles, so projections start as soon as chunk 0 lands). Dummy
    warm-up matmuls keep the PE busy meanwhile so it reaches 2.4 GHz.
    k[128, 4096], q[128, 2048] = W4 @ x + bias: weights are band-replicated
    4x on the host, so the projection itself fills all four 32-row bands
    (no replication DMAs); bias-adds alternate between DVE and ACT.
    vT[j][128, 257] per 128-key tile = (gamma*Wv) @ x; col 256 = 1.0 (the
    softmax denominator rides the attention matmul as an extra channel;
    gamma*bv collapses into the residual since softmax rows sum to 1).
    scores quad [128k, 4, 512q] = 4 concurrent 32-row tile_position MMs
    e = exp(scores - 40) -> bf16, one ACTIVATE per quad (N=2048, the exp
    chain is the pipeline backbone at ~2.66us/step)
    out[q, 0:257] += e-tile.T @ vT[j]      (PE, bf16, ~110ns/MM roofline)
    final[i, c] = out[i, c]/denom_i + (xT[i, c] + gamma*bv_c)
Output is stored [n, c]; the host transposes back to [c, n].
"""

import sys

if "/opt/trn_rl_repo" not in sys.path:
    sys.path.insert(0, "/opt/trn_rl_repo")

import numpy as np

import concourse.bass as bass
import concourse.tile as tile
from concourse import bacc
from concourse import mybir

F32 = mybir.dt.float32
BF16 = mybir.dt.bfloat16

C = 256          # channels
D = 32           # q/k channels
NK = 4096        # keys per core (full sequence)
NQ = 2048        # queries per core (half sequence)
NJ = NK // 128   # 32 key tiles
NG = 4           # query groups
GI = 4           # i-tiles (128 queries) per group
ISPAN = NQ // NG  # 512 query columns per group
NCH = 8          # x column chunks of 512
EXP_SHIFT = -40.0

Exp = mybir.ActivationFunctionType.Exp
Ident = mybir.ActivationFunctionType.Identity


# params_bf column layout (per partition p = one of 128 input-channel rows):
#   0:256    W4k  (h*128 + 32r + d)  -- Wk.T band-replicated 4x along M
#   256:512  W4q
#   512:1024 wv   (h*256 + c)
PW_K, PW_Q, PW_V = 0, 256, 512
PBF_COLS = 1024


def build(nc):
    x_bf = nc.declare_dram_parameter("x_bf", [C, NK], BF16, isOutput=False)
    xqT = nc.declare_dram_parameter("xqT", [NQ, C], F32, isOutput=False)
    params_bf = nc.declare_dram_parameter("params_bf", [128, PBF_COLS], BF16, isOutput=False)
    params_f32 = nc.declare_dram_parameter("params_f32", [128, 3], F32, isOutput=False)
    out_nc = nc.declare_dram_parameter("out_nc", [NQ, C], F32, isOutput=True)

    with tile.TileContext(nc) as tc:
        with (
            tc.tile_pool(name="singles", bufs=1) as singles,
            tc.tile_pool(name="epool", bufs=9) as e_pool,
            tc.tile_pool(name="osb", bufs=3) as osb_pool,
            tc.tile_pool(name="small", bufs=8) as small_pool,
            tc.tile_pool(name="s_ps", bufs=1, space="PSUM") as s_pool,
            tc.tile_pool(name="o_ps", bufs=4, space="PSUM") as o_pool,
        ):
            # ---------------- persistent SBUF inputs ----------------
            # All small weights ride ONE fast HWDGE transfer each; only the
            # late-needed xqT rides the slow gpsimd SWDGE path.
            pbf = singles.tile([128, PBF_COLS], BF16, name="params_bf")
            nc.scalar.dma_start(out=pbf, in_=params_bf[:, :])
            pf32 = singles.tile([128, 3], F32, name="params_f32")
            nc.scalar.dma_start(out=pf32, in_=params_f32[:, :])
            bk4_sb = pf32[:, 0:1]
            bq4_sb = pf32[:, 1:2]
            gamma_sb = pf32[:, 2:3]

            shift_sb = singles.tile([128, 1], F32)
            nc.vector.memset(shift_sb, EXP_SHIFT)

            # PE warm-up: dummy matmuls on memset data keep the PE busy from
            # program start until x chunk 0 lands (~4.5us), which both trips
            # the HAM un-throttle (needs ~3.4us of sustained busy) and avoids
            # the idle window that would re-throttle it -- so the projection
            # matmuls run at 2.4 GHz instead of 1.2.
            wu_src = singles.tile([128, 2, 512], BF16, name="wu")
            with tc.high_priority():
                nc.vector.memset(wu_src, 0.0)
                wu_ps = s_pool.tile([128, 4, ISPAN], F32, tag="ps_s", name="wu_ps")
                for i in range(10):
                    nc.tensor.matmul(
                        wu_ps[:, i % 4, :], wu_src[:, 0, 0:128], wu_src[:, 1, :],
                        start=True, stop=True,
                    )

            # x in 8 column chunks, ALL on the sync queue: within one queue
            # the descriptors drain in order, so chunk 0 completes ~0.7us
            # after issue instead of finishing together with all 2MB (the
            # SDMA engines round-robin across queues at packet granularity,
            # so splitting across queues destroys arrival ordering).
            x_r = x_bf.rearrange("(h p) n -> p h n", p=128)
            x_ch = [None] * NCH
            for cch in range(NCH):
                t = singles.tile([128, 2, 512], BF16, name=f"x{cch}")
                nc.sync.dma_start(out=t, in_=x_r[:, :, cch * 512 : (cch + 1) * 512])
                x_ch[cch] = t

            # xqT (residual, needed only at epilogues) queues behind the x
            # chunks so it never competes with them for HBM bandwidth.
            xqT_sb = singles.tile([128, NQ // 128, C], F32)
            nc.sync.dma_start(
                out=xqT_sb, in_=xqT.rearrange("(t p) c -> p t c", p=128)
            )

            # ---------------- k/q projections ----------------
            # Weights are band-replicated 4x on the host (W4), so one matmul
            # fills all four 32-row bands of k/q -- no replication DMAs.
            k_h = [
                singles.tile([128, NK // 2], BF16, name="k_h0"),
                singles.tile([128, NK // 2], BF16, name="k_h1"),
            ]
            q_sb = singles.tile([128, NQ], BF16)

            def kq_proj(w_off, b_sb, dst, dst_off, cch, slot):
                # one x chunk -> two 256-col psum slices -> bf16, all 128 rows.
                # Bias-adds alternate between DVE and the idle ACT engine.
                for s in range(2):
                    ps = o_pool.tile([128, C + 2], F32, tag="ps_o", name="ps_kq")
                    for h in range(2):
                        nc.tensor.matmul(
                            ps[:, 0:256],
                            pbf[:, w_off + h * 128 : w_off + (h + 1) * 128],
                            x_ch[cch][:, h, s * 256 : (s + 1) * 256],
                            start=(h == 0),
                            stop=(h == 1),
                        )
                    dsl = dst[:, dst_off + s * 256 : dst_off + (s + 1) * 256]
                    if (slot + s) % 2 == 0:
                        nc.vector.tensor_scalar_add(dsl, ps[:, 0:256], b_sb)
                    else:
                        nc.scalar.activation(
                            dsl, ps[:, 0:256], Ident, bias=b_sb, scale=1.0
                        )

            def kq_extra(m):
                # late k/q chunks, interleaved into the v-proj loop just
                # ahead of their first consumer
                if m < 4:
                    kq_proj(PW_K, bk4_sb, k_h[1], m * 512, m + 4, 0)
                elif m < 7:
                    kq_proj(PW_Q, bq4_sb, q_sb, (m - 3) * 512, m - 3, 1)

            # ---------------- v projection (per 128-key tile) ----------------
            # No v-bias on device: softmax weights sum to 1, so attn@(v+bv) =
            # attn@v + bv and the host folds gamma*bv into the residual xqT.
            # Column 256 of each vT tile (the softmax-denominator ones column)
            # is pre-memset here while the DVE is otherwise idle.
            vT = []
            for j in range(NJ):
                t = singles.tile([128, C + 1], BF16, name=f"vT{j}")
                nc.vector.memset(t[:, C : C + 1], 1.0)
                vT.append(t)

            def v_proj(j):
                cch, lj = j // 4, j % 4
                psv = o_pool.tile([128, C + 2], F32, tag="ps_o", name="ps_v")
                for h in range(2):
                    nc.tensor.matmul(
                        psv[:, 0:C],
                        x_ch[cch][:, h, lj * 128 : (lj + 1) * 128],
                        pbf[:, PW_V + h * C : PW_V + (h + 1) * C],
                        start=(h == 0),
                        stop=(h == 1),
                    )
                nc.vector.tensor_copy(vT[j][:, 0:C], psv[:, 0:C])

            # ---------------- attention ----------------
            steps = [(g, q4) for g in range(NG) for q4 in range(NJ // 4)]
            score_tiles = {}

            def emit_scores(step):
                g, q4 = step
                kh = k_h[q4 // 4]
                base = (q4 % 4) * 512
                ps_s = s_pool.tile([128, 4, ISPAN], F32, tag="ps_s", name="ps_s")
                for r in range(4):
                    nc.tensor.matmul(
                        ps_s[:, r, :],
                        kh[32 * r : 32 * (r + 1), base + r * 128 : base + (r + 1) * 128],
                        q_sb[32 * r : 32 * (r + 1), g * ISPAN : (g + 1) * ISPAN],
                        start=True,
                        stop=True,
                        tile_position=(32 * r, 0),
                    )
                e_sb = e_pool.tile([128, 4, ISPAN], BF16, tag="e_sb", name="e_sb")
                nc.scalar.activation(e_sb, ps_s, Exp, bias=shift_sb, scale=1.0)
                score_tiles[step] = e_sb

            def emit_tile_epilogue(g, t, ps_o):
                # gamma rides in Wv (host-folded), so f = ps_o/denom + xqT:
                # one reciprocal + one fused multiply-add, and the PSUM
                # accumulator frees as early as possible.
                it = g * GI + t
                r = small_pool.tile([128, 1], F32, tag="r", name="r")
                nc.vector.reciprocal(r, ps_o[t][:, C : C + 1])
                f_sb = osb_pool.tile([128, C], F32, tag="f_sb", name="f_sb")
                nc.vector.scalar_tensor_tensor(
                    f_sb,
                    ps_o[t][:, 0:C],
                    r,
                    xqT_sb[:, it, :],
                    op0=mybir.AluOpType.mult,
                    op1=mybir.AluOpType.add,
                )
                nc.sync.dma_start(out=out_nc[it * 128 : (it + 1) * 128, :], in_=f_sb)

            def emit_attn(step, ps_o):
                g, q4 = step
                e_sb = score_tiles.pop(step)
                if q4 == NJ // 4 - 1:
                    # last step of the group: finish tile-by-tile so each PSUM
                    # accumulator's epilogue starts (and the buffer frees for
                    # the next group) without waiting for the other tiles.
                    for t in range(GI):
                        for r in range(4):
                            j = q4 * 4 + r
                            nc.tensor.matmul(
                                ps_o[t][:, 0 : C + 1],
                                e_sb[:, r, t * 128 : (t + 1) * 128],
                                vT[j],
                                start=False,
                                stop=(j == NJ - 1),
                            )
                        emit_tile_epilogue(g, t, ps_o)
                elif q4 == 0:
                    # first step of a group: tile-major order, so tile t's
                    # first matmul waits only on tile t's buffer -- the
                    # previous group's epilogues free them one by one.
                    for t in range(GI):
                        for r in range(4):
                            nc.tensor.matmul(
                                ps_o[t][:, 0 : C + 1],
                                e_sb[:, r, t * 128 : (t + 1) * 128],
                                vT[r],
                                start=(r == 0),
                                stop=False,
                            )
                else:
                    for r in range(4):
                        j = q4 * 4 + r
                        for t in range(GI):
                            nc.tensor.matmul(
                                ps_o[t][:, 0 : C + 1],
                                e_sb[:, r, t * 128 : (t + 1) * 128],
                                vT[j],
                                start=(j == 0),
                                stop=(j == NJ - 1),
                            )

            # Group 0's scores need only q chunk 0 plus k chunk q4, so each of
            # the first four quads is emitted right behind the k-chunk it
            # consumes -- the exp chain starts as soon as chunk 0 lands.
            # The v-projections and remaining k/q chunks interleave after,
            # giving the ACT engine a LEAD-deep runway of e tiles.
            LEAD = 6
            with tc.high_priority():
                kq_proj(PW_K, bk4_sb, k_h[0], 0, 0, 0)
                kq_proj(PW_Q, bq4_sb, q_sb, 0, 0, 1)
                emit_scores(steps[0])
            for cch in range(1, 4):
                kq_proj(PW_K, bk4_sb, k_h[0], cch * 512, cch, 0)
                with tc.high_priority():
                    emit_scores(steps[cch])
            for m in range(8):
                v_proj(4 * m)
                v_proj(4 * m + 1)
                kq_extra(m)
                if m < 2:
                    emit_scores(steps[m + 4])
                v_proj(4 * m + 2)
                v_proj(4 * m + 3)
            # In-loop score emission runs at lead 2 (the PE parks on the next
            # pair right as its exp dependency clears); the LEAD-8 prefill
            # above is consumed over the first six steps so attention never
            # lags the exp chain by more than ~2 steps.
            ps_o_g = None
            for idx, (g, q4) in enumerate(steps):
                if idx + LEAD < len(steps):
                    emit_scores(steps[idx + LEAD])
                if q4 == 0:
                    ps_o_g = [
                        o_pool.tile([128, C + 2], F32, tag="ps_o", name="ps_o")
                        for _ in range(GI)
                    ]
                emit_attn((g, q4), ps_o_g)
    return nc


def _install_trace_support():
    """Profiling-only plumbing for KERNEL_TRACE=1 runs: register the NTFF
    profile hook (this image's antenv lacks the axon_hooks shim) and keep
    trace artifacts local instead of uploading. Never used in plain runs."""
    import importlib.util
    import types

    import concourse.bass_utils as bu

    bu.upload_artifacts = lambda tmpdir: tmpdir
    if "antenv.axon_hooks" in sys.modules:
        return
    try:
        if importlib.util.find_spec("antenv.axon_hooks") is not None:
            return
    except (ValueError, ModuleNotFoundError):
        return
    import antenv
    from trn_agent_boot.trn_boot import _ntff_profile_via_ctypes

    mod = types.ModuleType("antenv.axon_hooks")
    mod._hook = _ntff_profile_via_ctypes("/opt/axon/libaxon_pjrt.so")
    mod.set_axon_ntff_profile_hook = lambda h: setattr(mod, "_hook", h)
    mod.get_axon_ntff_profile_hook = lambda: mod._hook
    sys.modules["antenv.axon_hooks"] = mod
    antenv.axon_hooks = mod


_cached = None


def _get_module():
    global _cached
    if _cached is None:
        nc = bacc.Bacc()
        build(nc)
        if not nc.is_finalized():
            nc.finalize()
        _cached = nc
    return _cached


def kernel(x, Wq, bq, Wk, bk, Wv, bv, gamma, **_unused):
    from concourse.bass_utils import run_bass_kernel_spmd
    import os

    import ml_dtypes

    B, Cx, H, W = x.shape
    N = H * W
    xf = np.ascontiguousarray(np.asarray(x, dtype=np.float32).reshape(B, Cx, N))
    Wq = np.asarray(Wq, np.float32)
    Wk = np.asarray(Wk, np.float32)
    Wv = np.asarray(Wv, np.float32)
    bq = np.asarray(bq, np.float32)
    bk = np.asarray(bk, np.float32)
    bv = np.asarray(bv, np.float32)
    gamma = np.asarray(gamma, np.float32)

    # params_bf blob: see layout comment above build()
    pblob = np.zeros((128, PBF_COLS), np.float32)
    for h in range(2):
        for r in range(4):
            # W4k[p, h*128 + 32r + d] = Wk[d, h*128 + p]
            pblob[:, PW_K + h * 128 + 32 * r : PW_K + h * 128 + 32 * r + 32] = Wk[
                :, h * 128 : (h + 1) * 128
            ].T
            pblob[:, PW_Q + h * 128 + 32 * r : PW_Q + h * 128 + 32 * r + 32] = Wq[
                :, h * 128 : (h + 1) * 128
            ].T
        # wv[p, h*256 + c] = gamma*Wv[c, h*128 + p]  (gamma folded into Wv;
        # the softmax-denominator ones column stays unscaled)
        pblob[:, PW_V + h * C : PW_V + (h + 1) * C] = (
            gamma[0] * Wv[:, h * 128 : (h + 1) * 128].T
        )
    pblob_bf = np.ascontiguousarray(pblob.astype(ml_dtypes.bfloat16))
    pf32 = np.zeros((128, 3), np.float32)
    pf32[:, 0] = np.tile(bk, 4)
    pf32[:, 1] = np.tile(bq, 4)
    pf32[:, 2] = gamma[0]
    pf32 = np.ascontiguousarray(pf32)

    in_maps = []
    for core in range(8):
        b, half = core // 2, core % 2
        ioff = half * NQ
        xb = xf[b]
        x_roll = np.roll(xb, -ioff, axis=1)
        x_bf = np.ascontiguousarray(x_roll.astype(ml_dtypes.bfloat16))
        # residual + gamma*bv: softmax rows sum to 1, so the v-bias reduces to
        # a constant channel offset folded into the residual tensor.
        xqT_np = np.ascontiguousarray(xb[:, ioff : ioff + NQ].T + gamma[0] * bv[None, :])
        in_maps.append(
            {
                "x_bf": x_bf,
                "xqT": xqT_np,
                "params_bf": pblob_bf,
                "params_f32": pf32,
            }
        )

    nc = _get_module()
    trace = bool(int(os.environ.get("KERNEL_TRACE", "0")))
    if trace:
        _install_trace_support()
        tmpdir = os.environ.get("KERNEL_TRACE_DIR") or None
        res = run_bass_kernel_spmd(
            nc, in_maps, core_ids=list(range(8)), trace=True, tmpdir=tmpdir
        )
    else:
        res = run_bass_kernel_spmd(nc, in_maps, core_ids=list(range(8)))
    if trace and res.exec_time_ns is not None:
        print(f"HW exec time: {res.exec_time_ns} ns")
        print(f"HW exec time mean: {res.mean_exec_time_ns} ns")
        if res.instructions_and_trace is not None:
            print(f"trace: {res.instructions_and_trace[1]}")

    out = np.empty((B, Cx, N), np.float32)
    for core in range(8):
        b, half = core // 2, core % 2
        out[b][:, half * NQ : (half + 1) * NQ] = res.results[core]["out_nc"].T
    return out.reshape(B, Cx, H, W)


# revision 58
# speedup vs baseline: 1.0273x; 1.0063x over previous
"""Trainium2 Bass kernel for an AttentionBlock (1x1-conv QKV + softmax attention + residual).

Reference computation (per batch b):
    q = Wq@x + bq  [32, N];  k = Wk@x + bk  [32, N];  v = Wv@x + bv  [256, N]
    attn = softmax_j(q_i . k_j);  out[c, i] = sum_j v[c, j] attn[i, j]
    final = gamma * out + x            (N = 64*64 = 4096)

Sharding: 8 cores = 4 batches x 2 query-halves (2048 queries per core).
Each core receives x[b] with its columns rolled so its own query half sits at
columns 0:2048 (softmax is invariant to a permutation of the key/value axis).

Per-core device program (bf16 datapath; fp32 only for PSUM accum, the exp
input, the softmax reciprocal and the residual add, so the gamma=0 path
returns x exactly):
    x arrives bf16 in 8 column chunks on one DMA queue (in-order arrival;
    separate tiles, so projections start as soon as chunk 0 lands). Dummy
    warm-up matmuls keep the PE busy meanwhile so it reaches 2.4 GHz.
    k[128, 4096], q[128, 2048] = W4 @ x + bias: weights are band-replicated
    4x on the host, so the projection itself fills all four 32-row bands
    (no replication DMAs); bias-adds alternate between DVE and ACT.
    vT[j][128, 257] per 128-key tile = (gamma*Wv) @ x; col 256 = 1.0 (the
    softmax denominator rides the attention matmul as an extra channel;
    gamma*bv collapses into the residual since softmax rows sum to 1).
    scores quad [128k, 4, 512q] = 4 concurrent 32-row tile_position MMs
    e = exp(scores - 40) -> bf16, one ACTIVATE per quad (N=2048, the exp
    chain is the pipeline backbone at ~2.66us/step)
    out[q, 0:257] += e-tile.T @ vT[j]      (PE, bf16, ~110ns/MM roofline)
    final[i, c] = out[i, c]/denom_i + (xT[i, c] + gamma*bv_c)
Output is stored [n, c]; the host transposes back to [c, n].
"""

import sys

if "/opt/trn_rl_repo" not in sys.path:
    sys.path.insert(0, "/opt/trn_rl_repo")

import numpy as np

import concourse.bass as bass
import concourse.tile as tile
from concourse import bacc
from concourse import mybir

F32 = mybir.dt.float32
BF16 = mybir.dt.bfloat16

C = 256          # channels
D = 32           # q/k channels
NK = 4096        # keys per core (full sequence)
NQ = 2048        # queries per core (half sequence)
NJ = NK // 128   # 32 key tiles
NG = 4           # query groups
GI = 4           # i-tiles (128 queries) per group
ISPAN = NQ // NG  # 512 query columns per group
NCH = 8          # x column chunks of 512
EXP_SHIFT = -40.0

Exp = mybir.ActivationFunctionType.Exp
Ident = mybir.ActivationFunctionType.Identity


# params_bf column layout (per partition p = one of 128 input-channel rows):
#   0:256    W4k  (h*128 + 32r + d)  -- Wk.T band-replicated 4x along M
#   256:512  W4q
#   512:1024 wv   (h*256 + c)
PW_K, PW_Q, PW_V = 0, 256, 512
PBF_COLS = 1024


def build(nc):
    x_bf = nc.declare_dram_parameter("x_bf", [C, NK], BF16, isOutput=False)
    xqT = nc.declare_dram_parameter("xqT", [NQ, C], F32, isOutput=False)
    params_bf = nc.declare_dram_parameter("params_bf", [128, PBF_COLS], BF16, isOutput=False)
    params_f32 = nc.declare_dram_parameter("params_f32", [128, 3], F32, isOutput=False)
    out_nc = nc.declare_dram_parameter("out_nc", [NQ, C], F32, isOutput=True)

    with tile.TileContext(nc) as tc:
        with (
            tc.tile_pool(name="singles", bufs=1) as singles,
            tc.tile_pool(name="epool", bufs=9) as e_pool,
            tc.tile_pool(name="osb", bufs=3) as osb_pool,
            tc.tile_pool(name="small", bufs=8) as small_pool,
            tc.tile_pool(name="s_ps", bufs=1, space="PSUM") as s_pool,
            tc.tile_pool(name="o_ps", bufs=4, space="PSUM") as o_pool,
        ):
            # ---------------- persistent SBUF inputs ----------------
            # All small weights ride ONE fast HWDGE transfer each; only the
            # late-needed xqT rides the slow gpsimd SWDGE path.
            pbf = singles.tile([128, PBF_COLS], BF16, name="params_bf")
            nc.scalar.dma_start(out=pbf, in_=params_bf[:, :])
            pf32 = singles.tile([128, 3], F32, name="params_f32")
            nc.scalar.dma_start(out=pf32, in_=params_f32[:, :])
            bk4_sb = pf32[:, 0:1]
            bq4_sb = pf32[:, 1:2]
            gamma_sb = pf32[:, 2:3]

            shift_sb = singles.tile([128, 1], F32)
            nc.vector.memset(shift_sb, EXP_SHIFT)

            # PE warm-up: dummy matmuls on memset data keep the PE busy from
            # program start until x chunk 0 lands (~4.5us), which both trips
            # the HAM un-throttle (needs ~3.4us of sustained busy) and avoids
            # the idle window that would re-throttle it -- so the projection
            # matmuls run at 2.4 GHz instead of 1.2.
            wu_src = singles.tile([128, 2, 512], BF16, name="wu")
            nc.vector.memset(wu_src, 0.0)
            wu_ps = s_pool.tile([128, 4, ISPAN], F32, tag="ps_s", name="wu_ps")
            for i in range(10):
                nc.tensor.matmul(
                    wu_ps[:, i % 4, :], wu_src[:, 0, 0:128], wu_src[:, 1, :],
                    start=True, stop=True,
                )

            # x in 8 column chunks, ALL on the sync queue: within one queue
            # the descriptors drain in order, so chunk 0 completes ~0.7us
            # after issue instead of finishing together with all 2MB (the
            # SDMA engines round-robin across queues at packet granularity,
            # so splitting across queues destroys arrival ordering).
            x_r = x_bf.rearrange("(h p) n -> p h n", p=128)
            x_ch = [None] * NCH
            for cch in range(NCH):
                t = singles.tile([128, 2, 512], BF16, name=f"x{cch}")
                nc.sync.dma_start(out=t, in_=x_r[:, :, cch * 512 : (cch + 1) * 512])
                x_ch[cch] = t

            # xqT (residual, needed only at epilogues) queues behind the x
            # chunks so it never competes with them for HBM bandwidth.
            xqT_sb = singles.tile([128, NQ // 128, C], F32)
            nc.sync.dma_start(
                out=xqT_sb, in_=xqT.rearrange("(t p) c -> p t c", p=128)
            )

            # ---------------- k/q projections ----------------
            # Weights are band-replicated 4x on the host (W4), so one matmul
            # fills all four 32-row bands of k/q -- no replication DMAs.
            k_h = [
                singles.tile([128, NK // 2], BF16, name="k_h0"),
                singles.tile([128, NK // 2], BF16, name="k_h1"),
            ]
            q_sb = singles.tile([128, NQ], BF16)

            def kq_proj(w_off, b_sb, dst, dst_off, cch, slot):
                # one x chunk -> two 256-col psum slices -> bf16, all 128 rows.
                # Bias-adds alternate between DVE and the idle ACT engine.
                for s in range(2):
                    ps = o_pool.tile([128, C + 2], F32, tag="ps_o", name="ps_kq")
                    for h in range(2):
                        nc.tensor.matmul(
                            ps[:, 0:256],
                            pbf[:, w_off + h * 128 : w_off + (h + 1) * 128],
                            x_ch[cch][:, h, s * 256 : (s + 1) * 256],
                            start=(h == 0),
                            stop=(h == 1),
                        )
                    dsl = dst[:, dst_off + s * 256 : dst_off + (s + 1) * 256]
                    if (slot + s) % 2 == 0:
                        nc.vector.tensor_scalar_add(dsl, ps[:, 0:256], b_sb)
                    else:
                        nc.scalar.activation(
                            dsl, ps[:, 0:256], Ident, bias=b_sb, scale=1.0
                        )

            def kq_extra(m):
                # late k/q chunks, interleaved into the v-proj loop just
                # ahead of their first consumer
                if m < 4:
                    kq_proj(PW_K, bk4_sb, k_h[1], m * 512, m + 4, 0)
                elif m < 7:
                    kq_proj(PW_Q, bq4_sb, q_sb, (m - 3) * 512, m - 3, 1)

            # ---------------- v projection (per 128-key tile) ----------------
            # No v-bias on device: softmax weights sum to 1, so attn@(v+bv) =
            # attn@v + bv and the host folds gamma*bv into the residual xqT.
            # Column 256 of each vT tile (the softmax-denominator ones column)
            # is pre-memset here while the DVE is otherwise idle.
            vT = []
            for j in range(NJ):
                t = singles.tile([128, C + 1], BF16, name=f"vT{j}")
                nc.vector.memset(t[:, C : C + 1], 1.0)
                vT.append(t)

            def v_proj(j):
                cch, lj = j // 4, j % 4
                psv = o_pool.tile([128, C + 2], F32, tag="ps_o", name="ps_v")
                for h in range(2):
                    nc.tensor.matmul(
                        psv[:, 0:C],
                        x_ch[cch][:, h, lj * 128 : (lj + 1) * 128],
                        pbf[:, PW_V + h * C : PW_V + (h + 1) * C],
                        start=(h == 0),
                        stop=(h == 1),
                    )
                nc.vector.tensor_copy(vT[j][:, 0:C], psv[:, 0:C])

            # ---------------- attention ----------------
            steps = [(g, q4) for g in range(NG) for q4 in range(NJ // 4)]
            score_tiles = {}

            def emit_scores(step):
                g, q4 = step
                kh = k_h[q4 // 4]
                base = (q4 % 4) * 512
                ps_s = s_pool.tile([128, 4, ISPAN], F32, tag="ps_s", name="ps_s")
                for r in range(4):
                    nc.tensor.matmul(
                        ps_s[:, r, :],
                        kh[32 * r : 32 * (r + 1), base + r * 128 : base + (r + 1) * 128],
                        q_sb[32 * r : 32 * (r + 1), g * ISPAN : (g + 1) * ISPAN],
                        start=True,
                        stop=True,
                        tile_position=(32 * r, 0),
                    )
                e_sb = e_pool.tile([128, 4, ISPAN], BF16, tag="e_sb", name="e_sb")
                if step == steps[-1]:
                    # final step only: split the exp in halves so the last
                    # attention matmuls (and the DVE epilogue chain behind
                    # them) start one half-exp earlier; nothing follows this
                    # step, so the split cannot unbalance the pipeline.
                    for half in range(2):
                        nc.scalar.activation(
                            e_sb[:, 2 * half : 2 * half + 2, :],
                            ps_s[:, 2 * half : 2 * half + 2, :],
                            Exp,
                            bias=shift_sb,
                            scale=1.0,
                        )
                else:
                    nc.scalar.activation(e_sb, ps_s, Exp, bias=shift_sb, scale=1.0)
                score_tiles[step] = e_sb

            def emit_tile_epilogue(g, t, ps_o):
                # gamma rides in Wv (host-folded), so f = ps_o/denom + xqT:
                # one reciprocal + one fused multiply-add, and the PSUM
                # accumulator frees as early as possible.
                it = g * GI + t
                r = small_pool.tile([128, 1], F32, tag="r", name="r")
                nc.vector.reciprocal(r, ps_o[t][:, C : C + 1])
                f_sb = osb_pool.tile([128, C], F32, tag="f_sb", name="f_sb")
                nc.vector.scalar_tensor_tensor(
                    f_sb,
                    ps_o[t][:, 0:C],
                    r,
                    xqT_sb[:, it, :],
                    op0=mybir.AluOpType.mult,
                    op1=mybir.AluOpType.add,
                )
                nc.sync.dma_start(out=out_nc[it * 128 : (it + 1) * 128, :], in_=f_sb)

            def emit_attn(step, ps_o):
                g, q4 = step
                e_sb = score_tiles.pop(step)
                if q4 == NJ // 4 - 1:
                    # last step of the group: finish tile-by-tile so each PSUM
                    # accumulator's epilogue starts (and the buffer frees for
                    # the next group) without waiting for the other tiles.
                    for t in range(GI):
                        for r in range(4):
                            j = q4 * 4 + r
                            nc.tensor.matmul(
                                ps_o[t][:, 0 : C + 1],
                                e_sb[:, r, t * 128 : (t + 1) * 128],
                                vT[j],
                                start=False,
                                stop=(j == NJ - 1),
                            )
                        emit_tile_epilogue(g, t, ps_o)
                elif q4 == 0:
                    # first step of a group: tile-major order, so tile t's
                    # first matmul waits only on tile t's buffer -- the
                    # previous group's epilogues free them one by one.
                    for t in range(GI):
                        for r in range(4):
                            nc.tensor.matmul(
                                ps_o[t][:, 0 : C + 1],
                                e_sb[:, r, t * 128 : (t + 1) * 128],
                                vT[r],
                                start=(r == 0),
                                stop=False,
                            )
                else:
                    for r in range(4):
                        j = q4 * 4 + r
                        for t in range(GI):
                            nc.tensor.matmul(
                                ps_o[t][:, 0 : C + 1],
                                e_sb[:, r, t * 128 : (t + 1) * 128],
                                vT[j],
                                start=(j == 0),
                                stop=(j == NJ - 1),
                            )

            # Group 0's scores need only q chunk 0 plus k chunk q4, so each of
            # the first four quads is emitted right behind the k-chunk it
            # consumes -- the exp chain starts as soon as chunk 0 lands.
            # The v-projections and remaining k/q chunks interleave after,
            # giving the ACT engine a LEAD-deep runway of e tiles.
            LEAD = 6
            with tc.high_priority():
                kq_proj(PW_K, bk4_sb, k_h[0], 0, 0, 0)
                kq_proj(PW_Q, bq4_sb, q_sb, 0, 0, 1)
                emit_scores(steps[0])
            for cch in range(1, 4):
                kq_proj(PW_K, bk4_sb, k_h[0], cch * 512, cch, 0)
                with tc.high_priority():
                    emit_scores(steps[cch])
            for m in range(8):
                v_proj(4 * m)
                v_proj(4 * m + 1)
                kq_extra(m)
                if m < 2:
                    emit_scores(steps[m + 4])
                v_proj(4 * m + 2)
                v_proj(4 * m + 3)
            # In-loop score emission runs at lead 2 (the PE parks on the next
            # pair right as its exp dependency clears); the LEAD-8 prefill
            # above is consumed over the first six steps so attention never
            # lags the exp chain by more than ~2 steps.
            ps_o_g = None
            for idx, (g, q4) in enumerate(steps):
                if idx + LEAD < len(steps):
                    emit_scores(steps[idx + LEAD])
                if q4 == 0:
                    ps_o_g = [
                        o_pool.tile([128, C + 2], F32, tag="ps_o", name="ps_o")
                        for _ in range(GI)
                    ]
                emit_attn((g, q4), ps_o_g)
    return nc


def _install_trace_support():
    """Profiling-only plumbing for KERNEL_TRACE=1 runs: register the NTFF
    profile hook (this image's antenv lacks the axon_hooks shim) and keep
    trace artifacts local instead of uploading. Never used in plain runs."""
    import importlib.util
    import types

    import concourse.bass_utils as bu

    bu.upload_artifacts = lambda tmpdir: tmpdir
    if "antenv.axon_hooks" in sys.modules:
        return
    try:
        if importlib.util.find_spec("antenv.axon_hooks") is not None:
            return
    except (ValueError, ModuleNotFoundError):
        return
    import antenv
    from trn_agent_boot.trn_boot import _ntff_profile_via_ctypes

    mod = types.ModuleType("antenv.axon_hooks")
    mod._hook = _ntff_profile_via_ctypes("/opt/axon/libaxon_pjrt.so")
    mod.set_axon_ntff_profile_hook = lambda h: setattr(mod, "_hook", h)
    mod.get_axon_ntff_profile_hook = lambda: mod._hook
    sys.modules["antenv.axon_hooks"] = mod
    antenv.axon_hooks = mod


_cached = None


def _get_module():
    global _cached
    if _cached is None:
        nc = bacc.Bacc()
        build(nc)
        if not nc.is_finalized():
            nc.finalize()
        _cached = nc
    return _cached


def kernel(x, Wq, bq, Wk, bk, Wv, bv, gamma, **_unused):
    from concourse.bass_utils import run_bass_kernel_spmd
    import os

    import ml_dtypes

    B, Cx, H, W = x.shape
    N = H * W
    xf = np.ascontiguousarray(np.asarray(x, dtype=np.float32).reshape(B, Cx, N))
    Wq = np.asarray(Wq, np.float32)
    Wk = np.asarray(Wk, np.float32)
    Wv = np.asarray(Wv, np.float32)
    bq = np.asarray(bq, np.float32)
    bk = np.asarray(bk, np.float32)
    bv = np.asarray(bv, np.float32)
    gamma = np.asarray(gamma, np.float32)

    # params_bf blob: see layout comment above build()
    pblob = np.zeros((128, PBF_COLS), np.float32)
    for h in range(2):
        for r in range(4):
            # W4k[p, h*128 + 32r + d] = Wk[d, h*128 + p]
            pblob[:, PW_K + h * 128 + 32 * r : PW_K + h * 128 + 32 * r + 32] = Wk[
                :, h * 128 : (h + 1) * 128
            ].T
            pblob[:, PW_Q + h * 128 + 32 * r : PW_Q + h * 128 + 32 * r + 32] = Wq[
                :, h * 128 : (h + 1) * 128
            ].T
        # wv[p, h*256 + c] = gamma*Wv[c, h*128 + p]  (gamma folded into Wv;
        # the softmax-denominator ones column stays unscaled)
        pblob[:, PW_V + h * C : PW_V + (h + 1) * C] = (
            gamma[0] * Wv[:, h * 128 : (h + 1) * 128].T
        )
    pblob_bf = np.ascontiguousarray(pblob.astype(ml_dtypes.bfloat16))
    pf32 = np.zeros((128, 3), np.float32)
    pf32[:, 0] = np.tile(bk, 4)
    pf32[:, 1] = np.tile(bq, 4)
    pf32[:, 2] = gamma[0]
    pf32 = np.ascontiguousarray(pf32)

    in_maps = []
    for core in range(8):
        b, half = core // 2, core % 2
        ioff = half * NQ
        xb = xf[b]
        x_roll = np.roll(xb, -ioff, axis=1)
        x_bf = np.ascontiguousarray(x_roll.astype(ml_dtypes.bfloat16))
        # residual + gamma*bv: softmax rows sum to 1, so the v-bias reduces to
        # a constant channel offset folded into the residual tensor.
        xqT_np = np.ascontiguousarray(xb[:, ioff : ioff + NQ].T + gamma[0] * bv[None, :])
        in_maps.append(
            {
                "x_bf": x_bf,
                "xqT": xqT_np,
                "params_bf": pblob_bf,
                "params_f32": pf32,
            }
        )

    nc = _get_module()
    trace = bool(int(os.environ.get("KERNEL_TRACE", "0")))
    if trace:
        _install_trace_support()
        tmpdir = os.environ.get("KERNEL_TRACE_DIR") or None
        res = run_bass_kernel_spmd(
            nc, in_maps, core_ids=list(range(8)), trace=True, tmpdir=tmpdir
        )
    else:
        res = run_bass_kernel_spmd(nc, in_maps, core_ids=list(range(8)))
    if trace and res.exec_time_ns is not None:
        print(f"HW exec time: {res.exec_time_ns} ns")
        print(f"HW exec time mean: {res.mean_exec_time_ns} ns")
        if res.instructions_and_trace is not None:
            print(f"trace: {res.instructions_and_trace[1]}")

    out = np.empty((B, Cx, N), np.float32)
    for core in range(8):
        b, half = core // 2, core % 2
        out[b][:, half * NQ : (half + 1) * NQ] = res.results[core]["out_nc"].T
    return out.reshape(B, Cx, H, W)
